# revision 20
# baseline (speedup 1.0000x reference)
"""Paged-attention decode kernel for 8 Trainium2 NeuronCores.

Problem: B=32 decode sequences, GQA (32 q heads / 8 kv heads), head_dim=128,
paged KV cache of 2048 blocks x 16 tokens. Scatter new k/v tokens, then for
each sequence attend over its (up to 2048) cached tokens selected by a block
table.

v2 strategy (v1 in kernel_v1.py):
  - Host: apply the slot_mapping scatter, cast caches to bf16, sort sequences
    into 4 rank blocks of 8 and greedily balance blocks across cores so each
    core's total gathered tokens is near sum/8 = 3794 (the v1 rank-rigid
    assignment left a 4832 vs 3072 imbalance while every core gathered the
    full 5120-token bucket sum). Build per-core idx lists with -1 tails,
    per-window token counts, softmax masks, pre-scaled qT. K/V gather issue
    is interleaved per window so both SWDGE queues fill from slot start.
  - Device: K and V gathers in 512-token windows (gathers with num_idxs >
    512 fault the device - descriptor-ring limit). num_idxs_reg comes from
    runtime registers loaded from the "cnt" input, so only each sequence's
    actual ceil16(L) tokens move: worst core 3824 tokens (15.7 MB, after a
    swap-refinement pass) vs the
    uniform 5120 (21.0 MB) of v1. Gather tiles are memset on rep 0 so
    runtime-short gathers leave exact zeros (exp(0)*mask0 = 0; PV adds 0).
    Per (slot, head): QK^T chunk matmuls -> scores^T [tok, G] in PSUM,
    exp (no max subtraction: |scores| <= ~40 << 88), mask multiply, then
    PV^T matmuls accumulate o^T [d=128, G] (4-col streams instead of the v1
    128-col o[G, d] streams) and a ones-lhsT matmul accumulates softmax
    denominators into a [1, 128] PSUM tile. Normalization happens on host
    (outputs are o^T and den, both un-normalized).
"""

import os
import sys
from contextlib import ExitStack

import numpy as np

for _p in ("/opt/trn_rl_repo", "/root/.axon_site/_ro/trn_rl_repo"):
    if os.path.isdir(_p) and _p not in sys.path:
        sys.path.insert(0, _p)

import ml_dtypes  # noqa: E402

import concourse.bass as bass  # noqa: E402
from concourse import bacc  # noqa: E402
import concourse.tile as tile  # noqa: E402
from concourse import mybir  # noqa: E402

B = 32
NUM_BLOCKS = 2048
BLOCK_SIZE = 16
KVH = 8
NH = 32
D = 128
MAX_BLOCKS = 128
G = NH // KVH  # 4 q heads per kv head
ROWS = NUM_BLOCKS * BLOCK_SIZE  # 32768 flat cache rows
ROW_ELEMS = KVH * D  # 1024 elements per token row
SCALE = float(1.0 / np.sqrt(D))
N_CORES = 8
SLOTS = 4  # sequences per core
NQ = int(os.environ.get("KRN_NQ", "2"))  # SWDGE queues
KVBUFS = int(os.environ.get("KRN_KVBUFS", "9"))  # kt pool buffers
SCRATCH = int(os.environ.get("KRN_SCRATCH", "32768"))  # SWDGE descriptor ring
CHUNK = 128  # tokens per matmul chunk
STATIC = os.environ.get("KRN_STATIC", "0") == "1"  # debug: no runtime regs
SMAX = MAX_BLOCKS * BLOCK_SIZE  # 2048 max tokens per slot
WINK = int(os.environ.get("KRN_WINK", "512"))  # tokens per K gather window
WINV = int(os.environ.get("KRN_WINV", "512"))  # tokens per V gather window
VBUFS = int(os.environ.get("KRN_VBUFS", "9"))  # vt pool buffers
BF16 = mybir.dt.bfloat16
F32 = mybir.dt.float32

_prog_cache: dict = {}


def _build_program(buckets, repeat=1, mode="full"):
    """One SPMD program for all 8 cores; buckets[j] = padded token count of
    sequence slot j (multiple of CHUNK=128, sorted descending). Gathers are
    runtime-sized from the "cnt" input; compute is static over buckets.

    repeat > 1 duplicates the compute body for marginal-time benchmarking.
    mode: "full" | "gather" (no compute) | "compute" (host limits counts)."""
    nch = [b // CHUNK for b in buckets]
    nwk = [(b + WINK - 1) // WINK for b in buckets]
    nwv = [(b + WINV - 1) // WINV for b in buckets]
    wkoff = np.cumsum([0] + nwk).tolist()
    wvoff = np.cumsum([0] + nwv).tolist()
    NW = wkoff[-1] + wvoff[-1]
    mask_off = np.cumsum([0] + [n * G for n in nch]).tolist()
    MC = mask_off[-1]

    nc = bacc.Bacc(num_swdge_queues=NQ, dynamic_dma_scratch_size=SCRATCH)
    kc_d = nc.declare_dram_parameter("kc", [ROWS, ROW_ELEMS], BF16, isOutput=False)
    vc_d = nc.declare_dram_parameter("vc", [ROWS, ROW_ELEMS], BF16, isOutput=False)
    qT_d = nc.declare_dram_parameter("qT", [128, 128], BF16, isOutput=False)
    idx_d = nc.declare_dram_parameter(
        "idx", [128, SLOTS * (SMAX // 16)], mybir.dt.int16, isOutput=False
    )
    mask_d = nc.declare_dram_parameter("mask", [128, MC], BF16, isOutput=False)
    cnt_d = nc.declare_dram_parameter("cnt", [1, NW], mybir.dt.int32, isOutput=False)
    oT_d = nc.declare_dram_parameter("oT", [128, 128], F32, isOutput=True)
    den_d = nc.declare_dram_parameter("den", [1, SLOTS * KVH * G], F32, isOutput=True)

    with tile.TileContext(nc) as tc, ExitStack() as ctx:
        const = ctx.enter_context(tc.tile_pool(name="const", bufs=1))
        ktp = ctx.enter_context(tc.tile_pool(name="ktp", bufs=KVBUFS))
        vtp = ctx.enter_context(tc.tile_pool(name="vtp", bufs=VBUFS))
        ptp = ctx.enter_context(tc.tile_pool(name="ptp", bufs=4))
        scp = ctx.enter_context(tc.tile_pool(name="scp", bufs=3, space=bass.MemorySpace.PSUM))
        oap = ctx.enter_context(tc.tile_pool(name="oap", bufs=2, space=bass.MemorySpace.PSUM))
        dnp = ctx.enter_context(tc.tile_pool(name="dnp", bufs=2, space=bass.MemorySpace.PSUM))
        osbp = ctx.enter_context(tc.tile_pool(name="osbp", bufs=2))

        idx = const.tile([128, SLOTS * (SMAX // 16)], mybir.dt.int16)
        nc.sync.dma_start(idx[:], idx_d[:])
        qT = const.tile([128, 128], BF16)
        nc.sync.dma_start(qT[:], qT_d[:])
        mask = const.tile([128, MC], BF16)
        nc.sync.dma_start(mask[:], mask_d[:])
        cntt = const.tile([1, NW], mybir.dt.int32)
        nc.sync.dma_start(cntt[:], cnt_d[:])
        ones = const.tile([128, 1], BF16)
        nc.vector.memset(ones[:], 1.0)

        regs = [
            nc.values_load(
                cntt[0:1, w : w + 1],
                engines=[mybir.EngineType.Pool],
                skip_runtime_bounds_check=True,
            )
            for w in range(NW)
        ]

        for _rep in range(repeat):
            if mode != "gather":
                oT = oap.tile([128, 128], F32)
                den = dnp.tile([1, SLOTS * KVH * G], F32)
            oT_sb = osbp.tile([128, 256], F32)
            for j in range(SLOTS):
                kts, vts = [], []
                for w in range(max(nwk[j], nwv[j])):
                    if w < nwk[j]:
                        toks = min(WINK, buckets[j] - w * WINK)
                        i0 = j * (SMAX // 16) + w * (WINK // 16)
                        isl = idx[:, i0 : i0 + toks // 16]
                        reg = toks if STATIC else regs[wkoff[j] + w]
                        kt = ktp.tile([128, KVH, toks], BF16)
                        if _rep == 0:
                            # Zero once so runtime-short gathers leave exact
                            # zeros (exp(0)*mask0 = 0; PV on zeros adds 0).
                            nc.vector.memset(kt[:], 0.0)
                        nc.gpsimd.dma_gather(
                            kt[:], kc_d[:], isl,
                            num_idxs=toks, num_idxs_reg=reg, elem_size=ROW_ELEMS,
                            transpose=True, queue_num=w % NQ,
                        )
                        kts.append(kt)
                    if w < nwv[j]:
                        toks = min(WINV, buckets[j] - w * WINV)
                        i0 = j * (SMAX // 16) + w * (WINV // 16)
                        isl = idx[:, i0 : i0 + toks // 16]
                        reg = toks if STATIC else regs[wkoff[-1] + wvoff[j] + w]
                        vt = vtp.tile([128, toks // CHUNK, ROW_ELEMS], BF16)
                        if _rep == 0:
                            nc.vector.memset(vt[:], 0.0)
                        nc.gpsimd.dma_gather(
                            vt[:], vc_d[:], isl,
                            num_idxs=toks, num_idxs_reg=reg, elem_size=ROW_ELEMS,
                            transpose=False, queue_num=(w + 1) % NQ,
                        )
                        vts.append(vt)
                if mode == "gather":
                    continue
                WCK = WINK // CHUNK
                WCV = WINV // CHUNK
                for h in range(KVH):
                    row = j * KVH + h
                    qcol = row * G
                    sc = scp.tile([128, nch[j] * G], F32)
                    for c in range(nch[j]):
                        nc.tensor.matmul(
                            sc[:, c * G : (c + 1) * G],
                            kts[c // WCK][:, h, (c % WCK) * CHUNK : (c % WCK + 1) * CHUNK],
                            qT[:, qcol : qcol + G],
                            start=True, stop=True,
                        )
                    pt = ptp.tile([128, nch[j] * G], BF16)
                    nc.scalar.activation(pt[:], sc[:], mybir.ActivationFunctionType.Exp)
                    m0 = mask_off[j]
                    nc.vector.tensor_mul(pt[:], pt[:], mask[:, m0 : m0 + nch[j] * G])
                    for c in range(nch[j]):
                        nc.tensor.matmul(
                            oT[:, qcol : qcol + G],
                            vts[c // WCV][:, c % WCV, h * D : (h + 1) * D],
                            pt[:, c * G : (c + 1) * G],
                            start=(c == 0), stop=(c == nch[j] - 1),
                            skip_group_check=True,
                        )
                        nc.tensor.matmul(
                            den[0:1, qcol : qcol + G],
                            ones[:],
                            pt[:, c * G : (c + 1) * G],
                            start=(c == 0), stop=(c == nch[j] - 1),
                            skip_group_check=True,
                        )
            if mode == "gather":
                nc.vector.memset(oT_sb[:], 0.0)
                nc.sync.dma_start(oT_d[:], oT_sb[:, 0:128])
                nc.sync.dma_start(den_d[:], oT_sb[0:1, 128:256])
                continue
            nc.vector.tensor_copy(oT_sb[:, 0:128], oT[:])
            nc.vector.tensor_copy(oT_sb[0:1, 128:256], den[:])
            nc.sync.dma_start(oT_d[:], oT_sb[:, 0:128])
            nc.sync.dma_start(den_d[:], oT_sb[0:1, 128:256])
    nc.finalize()
    return nc


def _prep(q, k, v, k_cache, v_cache, context_lens, block_tables, slot_mapping):
    """Host-side prep: scatter, bf16 cast, balanced slot assignment, per-core
    operands (idx with -1 tails, runtime counts, masks, qT)."""
    lens = np.asarray(context_lens).astype(np.int64)
    bt = np.asarray(block_tables).astype(np.int64)
    sm = np.asarray(slot_mapping).astype(np.int64)

    kc = np.ascontiguousarray(np.asarray(k_cache, np.float32)).reshape(ROWS, ROW_ELEMS).copy()
    vc = np.ascontiguousarray(np.asarray(v_cache, np.float32)).reshape(ROWS, ROW_ELEMS).copy()
    kc[sm] = np.asarray(k, np.float32).reshape(B, ROW_ELEMS)
    vc[sm] = np.asarray(v, np.float32).reshape(B, ROW_ELEMS)
    kc16 = kc.astype(ml_dtypes.bfloat16)
    vc16 = vc.astype(ml_dtypes.bfloat16)

    srt = np.argsort(-lens, kind="stable")
    # Rank blocks of 8 keep the static buckets minimal; within each block,
    # give the longest remaining sequence to the least-loaded core.
    order = np.zeros(B, np.int64)  # order[j*N_CORES + n] = seq of core n slot j
    load = np.zeros(N_CORES, np.int64)
    for j in range(SLOTS):
        block = list(srt[j * N_CORES : (j + 1) * N_CORES])  # sorted desc
        free = set(range(N_CORES))
        for s in block:
            n = min(free, key=lambda c: (load[c], c))
            free.remove(n)
            order[j * N_CORES + n] = s
            load[n] += (lens[s] + 15) // 16 * 16
    # swap refinement: trade sequences within a rank block if it lowers the
    # worst-core load (the critical core sets the gather-transfer wall)
    c16 = (lens + 15) // 16 * 16
    for _ in range(8):
        improved = False
        for j in range(SLOTS):
            for a in range(N_CORES):
                for b in range(a + 1, N_CORES):
                    sa, sb = order[j * N_CORES + a], order[j * N_CORES + b]
                    d = c16[sa] - c16[sb]
                    if d == 0:
                        continue
                    new_a, new_b = load[a] - d, load[b] + d
                    if max(new_a, new_b) < max(load[a], load[b]):
                        order[j * N_CORES + a], order[j * N_CORES + b] = sb, sa
                        load[a], load[b] = new_a, new_b
                        improved = True
        if not improved:
            break
    buckets = tuple(
        max(CHUNK, int(np.ceil(lens[srt[j * N_CORES]] / CHUNK)) * CHUNK)
        for j in range(SLOTS)
    )
    nch = [b // CHUNK for b in buckets]
    nwk = [(b + WINK - 1) // WINK for b in buckets]
    nwv = [(b + WINV - 1) // WINV for b in buckets]
    wkoff = np.cumsum([0] + nwk).tolist()
    wvoff = np.cumsum([0] + nwv).tolist()
    NW = wkoff[-1] + wvoff[-1]
    MC = sum(n * G for n in nch)

    qs = (np.asarray(q, np.float32)[:, 0] * SCALE).reshape(B, NH, D)

    in_maps = []
    for n in range(N_CORES):
        qT = np.zeros((128, 128), np.float32)
        idxs = np.full((16, SLOTS * (SMAX // 16)), 0 if STATIC else -1, np.int16)
        msk = np.zeros((128, MC), np.float32)
        cnt = np.zeros((1, NW), np.int32)
        mo = 0
        for j in range(SLOTS):
            s = int(order[j * N_CORES + n])
            L = int(lens[s])
            c16 = (L + 15) // 16 * 16
            nb = c16 // 16
            io = j * (SMAX // 16)
            idxs[:, io : io + nb] = (
                bt[s, :nb][None, :] * BLOCK_SIZE + np.arange(16)[:, None]
            ).astype(np.int16)
            for w in range(nwk[j]):
                toks = min(WINK, buckets[j] - w * WINK)
                lo = w * WINK
                c_w = min(max(c16 - lo, 16), toks)
                cnt[0, wkoff[j] + w] = c_w
                if c16 - lo < 16:
                    # ensure >=16 valid entries per window (safe idx 0,
                    # masked out) so num_idxs_reg is never 0
                    idxs[:, io + lo // 16] = 0
            for w in range(nwv[j]):
                toks = min(WINV, buckets[j] - w * WINV)
                lo = w * WINV
                cnt[0, wkoff[-1] + wvoff[j] + w] = min(max(c16 - lo, 16), toks)
            valid = (
                np.arange(128)[:, None] + np.arange(nch[j])[None, :] * CHUNK
            ) < L  # [128, nch]
            msk[:, mo : mo + nch[j] * G] = np.repeat(valid.astype(np.float32), G, axis=1)
            qT[:, j * 32 : (j + 1) * 32] = qs[s].reshape(32, D).T
            mo += nch[j] * G
        in_maps.append(
            {
                "kc": kc16,
                "vc": vc16,
                "qT": qT.astype(ml_dtypes.bfloat16),
                "idx": np.ascontiguousarray(np.tile(idxs, (8, 1))),
                "mask": msk.astype(ml_dtypes.bfloat16),
                "cnt": cnt,
            }
        )
    return buckets, order, in_maps


def _assemble(order, core_outs):
    out = np.zeros((B, 1, NH, D), np.float32)
    for n in range(N_CORES):
        oT = np.asarray(core_outs[n]["oT"], np.float32)  # [128, (j,h,g)]
        den = np.asarray(core_outs[n]["den"], np.float32).reshape(-1)  # (j,h,g)
        for j in range(SLOTS):
            s = int(order[j * N_CORES + n])
            blk = oT[:, j * 32 : (j + 1) * 32]  # [d, (h,g)]
            d_blk = den[j * 32 : (j + 1) * 32]  # (h,g)
            out[s, 0, :, :] = (blk / d_blk[None, :]).T
    return out


def kernel(q, k, v, k_cache, v_cache, context_lens, block_tables, slot_mapping):
    from concourse.bass_utils import run_bass_kernel_spmd

    buckets, order, in_maps = _prep(
        q, k, v, k_cache, v_cache, context_lens, block_tables, slot_mapping
    )
    key = ("hw", buckets)
    if key not in _prog_cache:
        _prog_cache[key] = _build_program(buckets)
    nc = _prog_cache[key]
    res = run_bass_kernel_spmd(nc, in_maps, list(range(N_CORES)))
    return _assemble(order, res.results)
